# revision 39
# baseline (speedup 1.0000x reference)
"""Trainium2 Bass kernel for a dense transformer block (pre-LN MHA + FFN).

Reference computation (B=2, S=2048, E=768, H=12, D=64, FF=3072):
    res = x
    xn = LN(x, gamma, beta)
    q,k,v = xn @ wq.T, xn @ wk.T, xn @ wv.T          (per-head D=64)
    attn = causal_softmax(q k^T / sqrt(D)) v
    res = res + attn @ wo.T + bo
    y = LN(res, gamma, beta)
    out = res + gelu(y @ w1.T + b1) @ w2.T + b2

Sharding: 8 cores, token-parallel with BLOCK-INTERLEAVED causal balancing.
Cores 0-3 = batch 0, cores 4-7 = batch 1. Within a batch group, core j owns
query tiles {j, j+4, j+8, j+12} (of 16 tiles x 128 tokens), stored sorted.
K^T and V are AllGather'ed within the group (owner-major layout, so gathered
key tile (owner r, slot s) = global tile 4s+r).

Attention is causal-skipped with a core-UNIFORM program: query slot qs only
visits gathered key slots s <= qs (40 of 64 block-visits). Diagonal-slot
(s == qs) visits get an additive mask (0 / upper-tri -30000 / all -30000,
selected per-core by an input tensor) applied as a PE accumulate-matmul
before exp, so softmax is exact and no DVE mask multiply is needed.

gamma is folded into wq/wk/wv/w1 host-side; beta becomes per-feature biases
(applied on the ACT engine during Q/K PSUM->SBUF copies; V's bias folds into
bo, w1's into b1). The LN kernel is therefore pure normalize.

Engine split: PE matmuls/transposes/mask-adds, ACT exp/GELU/Q-K-bias copies,
DVE stats/reciprocals/residual adds/copies, Pool(gpsimd) LN-apply and
SBUF-side bias adds.
"""

import numpy as np

import concourse.bass as bass
import concourse.mybir as mybir
import concourse.tile as tile
from concourse import bacc
from concourse.bass_utils import run_bass_kernel_spmd
from concourse.masks import make_identity

F32 = mybir.dt.float32
F32R = mybir.dt.float32r
BF16 = mybir.dt.bfloat16
AF = mybir.ActivationFunctionType
ALU = mybir.AluOpType

DEBUG_OUTPUTS = False

import os
KHEADS = int(os.environ.get("KHEADS", "12"))        # timing experiments only
KSKIP_FFN = os.environ.get("KSKIP_FFN", "0") == "1"
KSKIP_MASK = os.environ.get("KSKIP_MASK", "0") == "1"

B, S, E, H, FF = 2, 2048, 768, 12, 3072
D = E // H                      # 64
NCORES = 8
T = B * S // NCORES             # 512 tokens per core
TN = T // 128                   # 4 token tiles per core
EK = E // 128                   # 6 feature chunks
FK = FF // 128                  # 24 hidden chunks
GROUP = NCORES // B             # 4 cores per batch
SB = S // 128                   # 16 key tiles per batch
EPS = 1e-5
SCALE = 1.0 / np.sqrt(D)
NEG = -30000.0

# s-major score groups: for gathered key slot s, query slots qs>=s form a
# contiguous suffix of width W=(TN-s)*128. Chunk regions are placed at
# bank-aligned-or-within offsets so no matmul output crosses a PSUM bank.
# (s, owners, psc region offsets)
CHUNKS3 = [
    (0, [0, 1], [0, 512]),
    (0, [2, 3], [0, 512]),
    (1, [0, 1], [0, 512]),
    (1, [2, 3], [0, 512]),
    (2, [0, 1, 2, 3], [0, 256, 512, 768]),
    (3, [0, 1, 2, 3], [0, 128, 256, 384]),
]


def _ln_stats(nc, pools, x_ap, eps_sb):
    """mean/rstd of a [128, 768] tile (free-axis LN). Returns (mv, rstd)."""
    stats = pools["stats"].tile([128, 3, 6], F32, tag="stats")
    mv = pools["stats"].tile([128, 2], F32, tag="mv")
    for g in range(3):
        nc.vector.bn_stats(out=stats[:, g, :], in_=x_ap[:, g * 256:(g + 1) * 256])
    nc.vector.bn_aggr(out=mv, in_=stats)
    rstd = pools["stats"].tile([128, 1], F32, tag="rstd")
    nc.scalar.activation(out=rstd, in_=mv[:, 1:2], func=AF.Sqrt, bias=eps_sb, scale=1.0)
    nc.vector.reciprocal(out=rstd, in_=rstd)
    return mv, rstd


def declare_io(nc):
    io = {}
    io["x_own"] = nc.dram_tensor("x_own", [T, E], F32, kind="ExternalInput").ap()
    for nm in ("wqT", "wkT", "wvT", "woT"):
        io[nm] = nc.dram_tensor(nm, [E, E], BF16, kind="ExternalInput").ap()
    io["w1T"] = nc.dram_tensor("w1T", [E, FF], BF16, kind="ExternalInput").ap()
    io["w2T"] = nc.dram_tensor("w2T", [FF, E], BF16, kind="ExternalInput").ap()
    io["b1rs"] = nc.dram_tensor("b1rs", [128, FK], F32, kind="ExternalInput").ap()
    io["bo_row"] = nc.dram_tensor("bo_row", [1, E], F32, kind="ExternalInput").ap()
    io["b2_row"] = nc.dram_tensor("b2_row", [1, E], F32, kind="ExternalInput").ap()
    io["bqT"] = nc.dram_tensor("bqT", [128, EK], F32, kind="ExternalInput").ap()
    io["bkT"] = nc.dram_tensor("bkT", [128, EK], F32, kind="ExternalInput").ap()
    io["masks"] = nc.dram_tensor("masks", [GROUP, 128, 128], BF16,
                                 kind="ExternalInput").ap()
    io["out"] = nc.dram_tensor("out", [T, E], F32, kind="ExternalOutput").ap()
    if DEBUG_OUTPUTS:
        for nm, shp, dt in (("dbg_xnT", [128, EK * T], BF16),
                            ("dbg_qT", [128, EK * T], BF16),
                            ("dbg_attnT", [128, EK * T], BF16),
                            ("dbg_res", [128, TN * E], F32),
                            ("dbg_hT", [128, FK * T], BF16)):
            io[nm] = nc.dram_tensor(nm, shp, dt, kind="ExternalOutput").ap()
    return io


def build_kernel_body(tc, io, skip_collectives=False):
    nc = tc.nc
    x_own, wqT, wkT, wvT, woT = (io[k] for k in ("x_own", "wqT", "wkT", "wvT", "woT"))
    w1T, w2T, b1rs = io["w1T"], io["w2T"], io["b1rs"]
    bo_row, b2_row = io["bo_row"], io["b2_row"]
    bqT_in, bkT_in, masks, out = io["bqT"], io["bkT"], io["masks"], io["out"]

    pools = {}
    ctx_pools = []

    def open_pool(name, **kw):
        cm = tc.tile_pool(name=name, **kw)
        pool = cm.__enter__()
        ctx_pools.append(cm)
        pool._cm = cm
        return pool

    persist = open_pool("persist", bufs=1)
    pools["stats"] = open_pool("stats", bufs=3)
    dram = open_pool("dram", bufs=1, space="DRAM")

    # ---- constants ----
    identity = persist.tile([128, 128], BF16)
    make_identity(nc, identity)

    ones_all = persist.tile([128, 128], F32)
    nc.vector.memset(ones_all, 1.0)

    eps_sb = persist.tile([128, 1], F32)
    nc.vector.memset(eps_sb, EPS)

    def rep128(name, row_ap):
        t = persist.tile([128, E], F32, name=name)
        src = bass.AP(tensor=row_ap.tensor, offset=row_ap.offset,
                      ap=[[0, 128]] + list(row_ap.ap[1:]))
        nc.sync.dma_start(out=t, in_=src)
        return t

    bo_rep = rep128("bo_rep", bo_row)
    b2_rep = rep128("b2_rep", b2_row)
    b1_sb = persist.tile([128, FK], F32)
    nc.sync.dma_start(out=b1_sb, in_=b1rs)
    bqT_sb = persist.tile([128, EK], F32)
    nc.sync.dma_start(out=bqT_sb, in_=bqT_in)
    bkT_sb = persist.tile([128, EK], F32)
    nc.sync.dma_start(out=bkT_sb, in_=bkT_in)
    mask01 = persist.tile([128, GROUP, 128], BF16)
    nc.sync.dma_start(out=mask01, in_=masks.rearrange("r p q -> p r q"))

    # ---- long-lived activations ----
    qT_sb = persist.tile([128, EK, T], BF16)      # q, feature-major, 2-head packed
    attnT_sb = persist.tile([128, EK, T], BF16)   # attention out, feature-major
    res_sb = persist.tile([128, TN, E], F32)      # post-attn residual, token-major
    x_sb = persist.tile([128, TN, E], F32)        # input x (token-major), reused
    # exp'd scores, region idx = s*GROUP+r holds [zeros(s*128) | exp suffix]
    ex_all = persist.tile([128, SB, T], BF16)

    # ---- AllGather bounce buffers ----
    ag_k_in = dram.tile([E, T], BF16)
    ag_v_in = dram.tile([T, E], BF16)
    ag_k_out = dram.tile([GROUP * E, T], BF16)
    ag_v_out = dram.tile([S, E], BF16)
    groups = [list(range(GROUP)), list(range(GROUP, NCORES))]

    # ================= Phase A: load x, LN1, transpose =================
    with tc.tile_pool(name="span_a", bufs=1) as pa, \
         tc.tile_pool(name="tr_a", bufs=2) as ptr, \
         tc.tile_pool(name="psum_ta", bufs=2, space="PSUM") as ppta:
        xr = x_own.rearrange("(n p) e -> p n e", p=128)
        for n in range(0, TN, 2):
            nc.sync.dma_start(out=x_sb[:, n:n + 2, :], in_=xr[:, n:n + 2, :])
        xnT_sb = pa.tile([128, EK, T], BF16)
        for n in range(TN):
            mv, rstd = _ln_stats(nc, pools, x_sb[:, n, :], eps_sb)
            xn_b = ptr.tile([128, E], BF16, tag="xnb")
            nc.gpsimd.tensor_scalar(
                out=xn_b, in0=x_sb[:, n, :], scalar1=mv[:, 0:1], scalar2=rstd,
                op0=ALU.subtract, op1=ALU.mult)
            tp = ppta.tile([128, E], BF16, tag="tp")
            for e in range(EK):
                nc.tensor.transpose(tp[:, e * 128:(e + 1) * 128],
                                    xn_b[:, e * 128:(e + 1) * 128], identity)
            nc.vector.tensor_copy(xnT_sb[:, :, n * 128:(n + 1) * 128],
                                  tp.rearrange("p (e t) -> p e t", e=EK))
        if DEBUG_OUTPUTS:
            nc.sync.dma_start(out=io["dbg_xnT"],
                              in_=xnT_sb.rearrange("p k t -> p (k t)"))

        # ================= Phase B: K, V, Q projections =================
        with tc.tile_pool(name="wproj", bufs=2) as pw, \
             tc.tile_pool(name="psum_b", bufs=3, space="PSUM") as pps:
            # K^T first (gates the AllGather), then V, local q^T last.
            wk_sb = pw.tile([128, EK, E], BF16, tag="w")
            wkr = wkT.rearrange("(k p) f -> p k f", p=128)
            for k in range(EK):
                nc.sync.dma_start(out=wk_sb[:, k, :], in_=wkr[:, k, :])
            for m in range(EK):
                ps = pps.tile([128, T], F32, tag="mm")
                for k in range(EK):
                    nc.tensor.matmul(ps, wk_sb[:, k, m * 128:(m + 1) * 128],
                                     xnT_sb[:, k, :], start=(k == 0),
                                     stop=(k == EK - 1))
                kcp = ptr.tile([128, T], BF16, tag="kcp")
                nc.scalar.add(out=kcp, in_=ps, add=bkT_sb[:, m:m + 1])
                nc.sync.dma_start(out=ag_k_in[m * 128:(m + 1) * 128, :], in_=kcp)

            # V: token-major [T, 768] (v bias folded into bo host-side)
            wv_sb = pw.tile([128, EK, E], BF16, tag="w")
            wvr = wvT.rearrange("(k p) f -> p k f", p=128)
            for k in range(EK):
                nc.sync.dma_start(out=wv_sb[:, k, :], in_=wvr[:, k, :])
            for n in range(TN):
                ps1 = pps.tile([128, 512], F32, tag="mm")
                ps2 = pps.tile([128, 256], F32, tag="mm2")
                for k in range(EK):
                    lhsT = xnT_sb[:, k, n * 128:(n + 1) * 128]
                    nc.tensor.matmul(ps1, lhsT, wv_sb[:, k, 0:512],
                                     start=(k == 0), stop=(k == EK - 1))
                    nc.tensor.matmul(ps2, lhsT, wv_sb[:, k, 512:768],
                                     start=(k == 0), stop=(k == EK - 1))
                vcp = ptr.tile([128, E], BF16, tag="vcp")
                nc.vector.tensor_copy(vcp[:, 0:512], ps1)
                nc.vector.tensor_copy(vcp[:, 512:768], ps2)
                nc.sync.dma_start(
                    out=ag_v_in.rearrange("(n p) e -> p n e", p=128)[:, n, :],
                    in_=vcp)

            # q^T: local only, overlaps the in-flight AllGathers
            wq_sb = pw.tile([128, EK, E], BF16, tag="w")
            wqr = wqT.rearrange("(k p) f -> p k f", p=128)
            for k in range(EK):
                nc.sync.dma_start(out=wq_sb[:, k, :], in_=wqr[:, k, :])
            for m in range(EK):
                ps = pps.tile([128, T], F32, tag="mm")
                for k in range(EK):
                    nc.tensor.matmul(ps, wq_sb[:, k, m * 128:(m + 1) * 128],
                                     xnT_sb[:, k, :], start=(k == 0),
                                     stop=(k == EK - 1))
                nc.scalar.add(out=qT_sb[:, m, :], in_=ps, add=bqT_sb[:, m:m + 1])

    if DEBUG_OUTPUTS:
        nc.sync.dma_start(out=io["dbg_qT"], in_=qT_sb.rearrange("p k t -> p (k t)"))

    # ================= Phase C: AllGather K^T and V =================
    if not skip_collectives:
        nc.gpsimd.collective_compute("AllGather", ALU.bypass,
                                     replica_groups=groups,
                                     ins=[ag_k_in[:]], outs=[ag_k_out[:]])
        nc.gpsimd.collective_compute("AllGather", ALU.bypass,
                                     replica_groups=groups,
                                     ins=[ag_v_in[:]], outs=[ag_v_out[:]])

    # ---- open late-phase pools early so weight DMAs overlap attention ----
    span_fgh = open_pool("span_fgh", bufs=1)
    hT_sb = span_fgh.tile([128, FK, T], BF16)     # FFN hidden, feature-major
    w1_sb = span_fgh.tile([128, EK, FF], BF16)
    w1r = w1T.rearrange("(k p) f -> p k f", p=128)
    for k in range(EK):
        for j in range(2):
            nc.sync.dma_start(out=w1_sb[:, k, j * 1536:(j + 1) * 1536],
                              in_=w1r[:, k, j * 1536:(j + 1) * 1536])
    w2_sb = span_fgh.tile([128, FK, E], BF16)
    w2r = w2T.rearrange("(k p) f -> p k f", p=128)
    for k in range(0, FK, 2):
        nc.sync.dma_start(out=w2_sb[:, k:k + 2, :], in_=w2r[:, k:k + 2, :])

    # bo pre-add into x (Pool, overlaps attention): res = x + bo + attn@woT
    for n in range(TN):
        nc.gpsimd.tensor_add(x_sb[:, n, :], x_sb[:, n, :], bo_rep)

    # ================= Phase D: attention =================
    kgr = ag_k_out.rearrange("(r hp p) t -> p r hp t", r=GROUP, hp=EK, p=128)
    vgr = ag_v_out.rearrange("(t p) e -> p t e", p=128)

    # zero-fill the padded prefix of each ex_all region (exp writes only the
    # suffix [s*128:T]; PV reads the full 512 cols)
    for s in range(1, TN):
        for r in range(GROUP):
            nc.gpsimd.memset(ex_all[:, s * GROUP + r, 0:s * 128], 0.0)

    with tc.tile_pool(name="attn_kv", bufs=2) as pkv, \
         tc.tile_pool(name="attn_v", bufs=3) as pv, \
         tc.tile_pool(name="attn_r", bufs=2) as pr, \
         tc.tile_pool(name="psum_s", bufs=2, space="PSUM") as pps_s, \
         tc.tile_pool(name="psum_a", bufs=2, space="PSUM") as pps_a:
        for h in range(KHEADS):
            hp, ho = h // 2, (h % 2) * 64
            if h % 2 == 0:
                khead2 = pkv.tile([128, GROUP, T], BF16, tag="khead")
                nc.sync.dma_start(out=khead2, in_=kgr[:, :, hp, :])
            vhead = pv.tile([128, SB, D + 1], BF16, tag="vhead")
            nc.sync.dma_start(out=vhead[:, :, 0:D],
                              in_=vgr[:, :, h * 64:(h + 1) * 64])
            nc.vector.memset(vhead[:, :, D:D + 1], 1.0)

            ps_attn = pps_a.tile([D + 1, T], F32, tag="pattn")
            qh = qT_sb[ho:ho + 64, hp, :]

            def strided(tile2d, col, pitch, cnt, w):
                base = tile2d[:, col:col + w]
                return bass.AP(tensor=base.tensor, offset=base.offset,
                               ap=[list(base.ap[0]), [pitch, cnt], [1, w]])

            for s, rs, offs in CHUNKS3:
                W = (TN - s) * 128
                n = len(rs)
                idx0 = s * GROUP + rs[0]
                psc = pps_s.tile([128, 1024], F32, tag="psc")
                for i, r in enumerate(rs):
                    nc.tensor.matmul(psc[:, offs[i]:offs[i] + W],
                                     khead2[ho:ho + 64, r, s * 128:(s + 1) * 128],
                                     qh[:, s * 128:T],
                                     start=True, stop=True)
                src = (psc[:, 0:W] if n == 1
                       else strided(psc, 0, offs[1] - offs[0], n, W))
                ex_out = ex_all[:, idx0:idx0 + n, s * 128:T]
                nc.scalar.activation(out=ex_out, in_=src,
                                     func=AF.Exp, scale=SCALE)
                # multiplicative 0/1 causal mask on the diagonal-slot blocks
                # (Pool engine, SBUF-only, off the PE critical path)
                if not KSKIP_MASK:
                    dex = ex_all[:, idx0:idx0 + n, s * 128:(s + 1) * 128]
                    nc.gpsimd.tensor_mul(dex, dex, mask01[:, rs[0]:rs[0] + n, :])

            for s in range(TN):
                for r in range(GROUP):
                    nc.tensor.matmul(ps_attn, vhead[:, r * TN + s, :],
                                     ex_all[:, s * GROUP + r, :],
                                     start=(s == 0 and r == 0),
                                     stop=(s == TN - 1 and r == GROUP - 1))

            # normalize: attnT[d, t] = ps_attn[d, t] / ps_attn[64, t]
            r_row = pr.tile([1, T], F32, tag="rrow")
            nc.vector.reciprocal(out=r_row, in_=ps_attn[D:D + 1, :])
            r64 = pr.tile([D, T], F32, tag="r64")
            nc.gpsimd.partition_broadcast(r64, r_row)
            nc.vector.tensor_mul(attnT_sb[ho:ho + 64, hp, :],
                                 ps_attn[0:D, :], r64)

    if DEBUG_OUTPUTS:
        nc.sync.dma_start(out=io["dbg_attnT"],
                          in_=attnT_sb.rearrange("p k t -> p (k t)"))

    # ================= Phase E: out-proj + residual + LN2 =================
    span_fg = open_pool("span_fg", bufs=1)
    y2nT_sb = span_fg.tile([128, EK, T], BF16)    # LN2 output, feature-major
    with tc.tile_pool(name="span_e", bufs=1) as pf, \
         tc.tile_pool(name="tr_e", bufs=2) as ptrf, \
         tc.tile_pool(name="psum_te", bufs=2, space="PSUM") as pptf, \
         tc.tile_pool(name="psum_e", bufs=3, space="PSUM") as ppsf:
        wo_sb = pf.tile([128, EK, E], BF16)
        wor = woT.rearrange("(k p) f -> p k f", p=128)
        for k in range(EK):
            nc.sync.dma_start(out=wo_sb[:, k, :], in_=wor[:, k, :])
        for n in range(TN):
            ps1 = ppsf.tile([128, 512], F32, tag="mm")
            ps2 = ppsf.tile([128, 256], F32, tag="mm2")
            for k in range(EK):
                lhsT = attnT_sb[:, k, n * 128:(n + 1) * 128]
                nc.tensor.matmul(ps1, lhsT, wo_sb[:, k, 0:512],
                                 start=(k == 0), stop=(k == EK - 1))
                nc.tensor.matmul(ps2, lhsT, wo_sb[:, k, 512:768],
                                 start=(k == 0), stop=(k == EK - 1))
            nc.vector.tensor_add(res_sb[:, n, 0:512], ps1, x_sb[:, n, 0:512])
            nc.vector.tensor_add(res_sb[:, n, 512:768], ps2, x_sb[:, n, 512:768])
            mv, rstd = _ln_stats(nc, pools, res_sb[:, n, :], eps_sb)
            y2n_b = ptrf.tile([128, E], BF16, tag="y2nb")
            nc.gpsimd.tensor_scalar(
                out=y2n_b, in0=res_sb[:, n, :], scalar1=mv[:, 0:1], scalar2=rstd,
                op0=ALU.subtract, op1=ALU.mult)
            tp = pptf.tile([128, E], BF16, tag="tp")
            for e in range(EK):
                nc.tensor.transpose(tp[:, e * 128:(e + 1) * 128],
                                    y2n_b[:, e * 128:(e + 1) * 128], identity)
            nc.vector.tensor_copy(y2nT_sb[:, :, n * 128:(n + 1) * 128],
                                  tp.rearrange("p (e t) -> p e t", e=EK))
            # pre-add b2 into res (Pool) for the final residual
            nc.gpsimd.tensor_add(res_sb[:, n, :], res_sb[:, n, :], b2_rep)

    if DEBUG_OUTPUTS:
        nc.sync.dma_start(out=io["dbg_res"], in_=res_sb.rearrange("p n e -> p (n e)"))

    # ================= Phase F: FFN1 + GELU -> hT =================
    with tc.tile_pool(name="psum_f", bufs=4, space="PSUM") as ppsg:
        for m in range(FK if not KSKIP_FFN else 2):
            ps = ppsg.tile([128, T], F32, tag="mm")
            kf1 = EK if not KSKIP_FFN else 1
            for k in range(kf1):
                nc.tensor.matmul(ps, w1_sb[:, k, m * 128:(m + 1) * 128],
                                 y2nT_sb[:, k, :], start=(k == 0),
                                 stop=(k == kf1 - 1))
            nc.scalar.activation(out=hT_sb[:, m, :], in_=ps, func=AF.Gelu,
                                 bias=b1_sb[:, m:m + 1], scale=1.0)

    if DEBUG_OUTPUTS:
        nc.sync.dma_start(out=io["dbg_hT"], in_=hT_sb.rearrange("p k t -> p (k t)"))

    # close span_fg (y2nT dead)
    ctx_pools.remove(span_fg._cm)
    span_fg._cm.__exit__(None, None, None)

    # ================= Phase G: FFN2 + residual -> out =================
    with tc.tile_pool(name="psum_g", bufs=3, space="PSUM") as ppsh:
        for n in range(TN):
            ps1 = ppsh.tile([128, 512], F32, tag="mm")
            ps2 = ppsh.tile([128, 256], F32, tag="mm2")
            kf2 = FK if not KSKIP_FFN else 2
            for k in range(kf2):
                lhsT = hT_sb[:, k, n * 128:(n + 1) * 128]
                nc.tensor.matmul(ps1, lhsT, w2_sb[:, k, 0:512],
                                 start=(k == 0), stop=(k == kf2 - 1))
                nc.tensor.matmul(ps2, lhsT, w2_sb[:, k, 512:768],
                                 start=(k == 0), stop=(k == kf2 - 1))
            nc.vector.tensor_add(res_sb[:, n, 0:512], res_sb[:, n, 0:512], ps1)
            nc.vector.tensor_add(res_sb[:, n, 512:768], res_sb[:, n, 512:768], ps2)
        outr = out.rearrange("(n p) e -> p n e", p=128)
        for n in range(0, TN, 2):
            nc.sync.dma_start(out=outr[:, n:n + 2, :], in_=res_sb[:, n:n + 2, :])

    for p in reversed(ctx_pools):
        p.__exit__(None, None, None)


_CACHED = {}


def _get_module(repeat=1, loop_n=None, skip_collectives=False):
    key = ("nc", repeat, loop_n, skip_collectives)
    if key not in _CACHED:
        nc = bacc.Bacc("TRN2", target_bir_lowering=False, debug=False,
                       enable_asserts=False, num_devices=NCORES)
        io = declare_io(nc)
        with tile.TileContext(nc) as tc:
            if loop_n is not None:
                with tc.For_i(0, loop_n, 1):
                    build_kernel_body(tc, io, skip_collectives=True)
            else:
                for _ in range(repeat):
                    build_kernel_body(tc, io, skip_collectives=skip_collectives)
        nc.compile()
        _CACHED[key] = nc
    return _CACHED[key]


def make_in_maps(x, wq, wk, wv, wo, bo, w1, b1, w2, b2, gamma, beta):
    import ml_dtypes
    f = np.float32
    bf = ml_dtypes.bfloat16
    xf = np.asarray(x, f)
    wq_, wk_, wv_, wo_ = (np.asarray(w, f) for w in (wq, wk, wv, wo))
    w1_, w2_ = np.asarray(w1, f), np.asarray(w2, f)
    gam, bet = np.asarray(gamma, f), np.asarray(beta, f)
    bo_, b1_, b2_ = np.asarray(bo, f), np.asarray(b1, f), np.asarray(b2, f)

    # gamma folds into weight columns; beta becomes per-output biases
    bq = wq_ @ bet
    bk = wk_ @ bet
    bv = wv_ @ bet
    bo_eff = bo_ + wo_ @ bv
    b1_eff = b1_ + w1_ @ bet

    common = {
        "wqT": np.ascontiguousarray((wq_.T * gam[:, None]).astype(bf)),
        "wkT": np.ascontiguousarray((wk_.T * gam[:, None]).astype(bf)),
        "wvT": np.ascontiguousarray((wv_.T * gam[:, None]).astype(bf)),
        "woT": np.ascontiguousarray(wo_.T.astype(bf)),
        "w1T": np.ascontiguousarray((w1_.T * gam[:, None]).astype(bf)),
        "w2T": np.ascontiguousarray(w2_.T.astype(bf)),
        "b1rs": np.ascontiguousarray(b1_eff.reshape(FK, 128).T),
        "bo_row": bo_eff.reshape(1, E),
        "b2_row": b2_.reshape(1, E),
        "bqT": np.ascontiguousarray(bq.reshape(EK, 128).T),
        "bkT": np.ascontiguousarray(bk.reshape(EK, 128).T),
    }
    # multiplicative diagonal-slot masks [k, q]:
    # r<j -> 1, r==j -> tri(k<=q), r>j -> 0
    kidx = np.arange(128)[:, None]
    qidx = np.arange(128)[None, :]
    tri = (kidx <= qidx).astype(f)
    in_maps = []
    for c in range(NCORES):
        b, j = c // GROUP, c % GROUP
        tiles = [j, j + 4, j + 8, j + 12]
        m = dict(common)
        m["x_own"] = np.ascontiguousarray(
            np.concatenate([xf[b, t * 128:(t + 1) * 128, :] for t in tiles]))
        mk = np.zeros((GROUP, 128, 128), f)
        mk[0:j] = 1.0
        mk[j] = tri
        m["masks"] = mk.astype(bf)
        in_maps.append(m)
    return in_maps


def run(in_maps, trace=False):
    nc = _get_module()
    return run_bass_kernel_spmd(nc, in_maps, core_ids=list(range(NCORES)),
                                trace=trace)


def kernel(x, wq, wk, wv, wo, bo, w1, b1, w2, b2, gamma, beta):
    in_maps = make_in_maps(x, wq, wk, wv, wo, bo, w1, b1, w2, b2, gamma, beta)
    res = run(in_maps)
    out = np.zeros((B, S, E), np.float32)
    for c in range(NCORES):
        b, j = c // GROUP, c % GROUP
        chunk = res.results[c]["out"]
        for slot, t in enumerate([j, j + 4, j + 8, j + 12]):
            out[b, t * 128:(t + 1) * 128] = chunk[slot * 128:(slot + 1) * 128]
    return out


# revision 41
# speedup vs baseline: 1.7262x; 1.7262x over previous
"""Trainium2 Bass kernel for a dense transformer block (pre-LN MHA + FFN).

Reference computation (B=2, S=2048, E=768, H=12, D=64, FF=3072):
    res = x
    xn = LN(x, gamma, beta)
    q,k,v = xn @ wq.T, xn @ wk.T, xn @ wv.T          (per-head D=64)
    attn = causal_softmax(q k^T / sqrt(D)) v
    res = res + attn @ wo.T + bo
    y = LN(res, gamma, beta)
    out = res + gelu(y @ w1.T + b1) @ w2.T + b2

Sharding: 8 cores, token-parallel with BLOCK-INTERLEAVED causal balancing.
Cores 0-3 = batch 0, cores 4-7 = batch 1. Within a batch group, core j owns
query tiles {j, j+4, j+8, j+12} (of 16 tiles x 128 tokens), stored sorted.
K^T and V are AllGather'ed within the group (owner-major layout, so gathered
key tile (owner r, slot s) = global tile 4s+r).

Attention is causal-skipped with a core-UNIFORM program: query slot qs only
visits gathered key slots s <= qs (40 of 64 block-visits). Diagonal-slot
(s == qs) visits get an additive mask (0 / upper-tri -30000 / all -30000,
selected per-core by an input tensor) applied as a PE accumulate-matmul
before exp, so softmax is exact and no DVE mask multiply is needed.

gamma is folded into wq/wk/wv/w1 host-side; beta becomes per-feature biases
(applied on the ACT engine during Q/K PSUM->SBUF copies; V's bias folds into
bo, w1's into b1). The LN kernel is therefore pure normalize.

Engine split: PE matmuls/transposes/mask-adds, ACT exp/GELU/Q-K-bias copies,
DVE stats/reciprocals/residual adds/copies, Pool(gpsimd) LN-apply and
SBUF-side bias adds.
"""

import numpy as np

import concourse.bass as bass
import concourse.mybir as mybir
import concourse.tile as tile
from concourse import bacc
from concourse.bass_utils import run_bass_kernel_spmd
from concourse.masks import make_identity

F32 = mybir.dt.float32
F32R = mybir.dt.float32r
BF16 = mybir.dt.bfloat16
AF = mybir.ActivationFunctionType
ALU = mybir.AluOpType

DEBUG_OUTPUTS = False

import os
KHEADS = int(os.environ.get("KHEADS", "12"))        # timing experiments only
KSKIP_FFN = os.environ.get("KSKIP_FFN", "0") == "1"
KSKIP_MASK = os.environ.get("KSKIP_MASK", "0") == "1"
KPV40 = os.environ.get("KPV40", "0") == "1"         # per-qs-block PV chains

B, S, E, H, FF = 2, 2048, 768, 12, 3072
D = E // H                      # 64
NCORES = 8
T = B * S // NCORES             # 512 tokens per core
TN = T // 128                   # 4 token tiles per core
EK = E // 128                   # 6 feature chunks
FK = FF // 128                  # 24 hidden chunks
GROUP = NCORES // B             # 4 cores per batch
SB = S // 128                   # 16 key tiles per batch
EPS = 1e-5
SCALE = 1.0 / np.sqrt(D)
NEG = -30000.0

# s-major score groups: for gathered key slot s, query slots qs>=s form a
# contiguous suffix of width W=(TN-s)*128. Chunk regions are placed at
# bank-aligned-or-within offsets so no matmul output crosses a PSUM bank.
# (s, owners, psc region offsets)
CHUNKS3 = [
    (0, [0, 1], [0, 512]),
    (0, [2, 3], [0, 512]),
    (1, [0, 1], [0, 512]),
    (1, [2, 3], [0, 512]),
    (2, [0, 1, 2, 3], [0, 256, 512, 768]),
    (3, [0, 1, 2, 3], [0, 128, 256, 384]),
]


def _ln_stats(nc, pools, x_ap, eps_sb):
    """mean/rstd of a [128, 768] tile (free-axis LN). Returns (mv, rstd)."""
    stats = pools["stats"].tile([128, 3, 6], F32, tag="stats")
    mv = pools["stats"].tile([128, 2], F32, tag="mv")
    for g in range(3):
        nc.vector.bn_stats(out=stats[:, g, :], in_=x_ap[:, g * 256:(g + 1) * 256])
    nc.vector.bn_aggr(out=mv, in_=stats)
    rstd = pools["stats"].tile([128, 1], F32, tag="rstd")
    nc.scalar.activation(out=rstd, in_=mv[:, 1:2], func=AF.Sqrt, bias=eps_sb, scale=1.0)
    nc.vector.reciprocal(out=rstd, in_=rstd)
    return mv, rstd


def declare_io(nc):
    io = {}
    io["x_own"] = nc.dram_tensor("x_own", [T, E], F32, kind="ExternalInput").ap()
    for nm in ("wqT", "wkT", "wvT", "woT"):
        io[nm] = nc.dram_tensor(nm, [E, E], BF16, kind="ExternalInput").ap()
    io["w1T"] = nc.dram_tensor("w1T", [E, FF], BF16, kind="ExternalInput").ap()
    io["w2T"] = nc.dram_tensor("w2T", [FF, E], BF16, kind="ExternalInput").ap()
    io["b1rs"] = nc.dram_tensor("b1rs", [128, FK], F32, kind="ExternalInput").ap()
    io["bo_row"] = nc.dram_tensor("bo_row", [1, E], F32, kind="ExternalInput").ap()
    io["b2_row"] = nc.dram_tensor("b2_row", [1, E], F32, kind="ExternalInput").ap()
    io["bqT"] = nc.dram_tensor("bqT", [128, EK], F32, kind="ExternalInput").ap()
    io["bkT"] = nc.dram_tensor("bkT", [128, EK], F32, kind="ExternalInput").ap()
    io["masks"] = nc.dram_tensor("masks", [GROUP, 128, 128], BF16,
                                 kind="ExternalInput").ap()
    io["out"] = nc.dram_tensor("out", [T, E], F32, kind="ExternalOutput").ap()
    if DEBUG_OUTPUTS:
        for nm, shp, dt in (("dbg_xnT", [128, EK * T], BF16),
                            ("dbg_qT", [128, EK * T], BF16),
                            ("dbg_attnT", [128, EK * T], BF16),
                            ("dbg_res", [128, TN * E], F32),
                            ("dbg_hT", [128, FK * T], BF16)):
            io[nm] = nc.dram_tensor(nm, shp, dt, kind="ExternalOutput").ap()
    return io


def build_kernel_body(tc, io, skip_collectives=False):
    nc = tc.nc
    x_own, wqT, wkT, wvT, woT = (io[k] for k in ("x_own", "wqT", "wkT", "wvT", "woT"))
    w1T, w2T, b1rs = io["w1T"], io["w2T"], io["b1rs"]
    bo_row, b2_row = io["bo_row"], io["b2_row"]
    bqT_in, bkT_in, masks, out = io["bqT"], io["bkT"], io["masks"], io["out"]

    pools = {}
    ctx_pools = []

    def open_pool(name, **kw):
        cm = tc.tile_pool(name=name, **kw)
        pool = cm.__enter__()
        ctx_pools.append(cm)
        pool._cm = cm
        return pool

    persist = open_pool("persist", bufs=1)
    pools["stats"] = open_pool("stats", bufs=3)
    dram = open_pool("dram", bufs=1, space="DRAM")

    # ---- constants ----
    identity = persist.tile([128, 128], BF16)
    make_identity(nc, identity)

    ones_all = persist.tile([128, 128], F32)
    nc.vector.memset(ones_all, 1.0)

    eps_sb = persist.tile([128, 1], F32)
    nc.vector.memset(eps_sb, EPS)

    def rep128(name, row_ap):
        t = persist.tile([128, E], F32, name=name)
        src = bass.AP(tensor=row_ap.tensor, offset=row_ap.offset,
                      ap=[[0, 128]] + list(row_ap.ap[1:]))
        nc.sync.dma_start(out=t, in_=src)
        return t

    bo_rep = rep128("bo_rep", bo_row)
    b2_rep = rep128("b2_rep", b2_row)
    b1_sb = persist.tile([128, FK], F32)
    nc.sync.dma_start(out=b1_sb, in_=b1rs)
    bqT_sb = persist.tile([128, EK], F32)
    nc.sync.dma_start(out=bqT_sb, in_=bqT_in)
    bkT_sb = persist.tile([128, EK], F32)
    nc.sync.dma_start(out=bkT_sb, in_=bkT_in)
    mask01 = persist.tile([128, GROUP, 128], BF16)
    nc.sync.dma_start(out=mask01, in_=masks.rearrange("r p q -> p r q"))

    # ---- long-lived activations ----
    qT_sb = persist.tile([128, EK, T], BF16)      # q, feature-major, 2-head packed
    attnT_sb = persist.tile([128, EK, T], BF16)   # attention out, feature-major
    res_sb = persist.tile([128, TN, E], F32)      # post-attn residual, token-major
    x_sb = persist.tile([128, TN, E], F32)        # input x (token-major), reused
    # exp'd scores, region idx = s*GROUP+r holds [zeros(s*128) | exp suffix]
    ex_all = persist.tile([128, SB, T], BF16)

    # ---- AllGather bounce buffers ----
    ag_k_in = dram.tile([E, T], BF16)
    ag_v_in = dram.tile([T, E], BF16)
    ag_k_out = dram.tile([GROUP * E, T], BF16)
    ag_v_out = dram.tile([S, E], BF16)
    groups = [list(range(GROUP)), list(range(GROUP, NCORES))]

    # ================= Phase A: load x, LN1, transpose =================
    with tc.tile_pool(name="span_a", bufs=1) as pa, \
         tc.tile_pool(name="tr_a", bufs=2) as ptr, \
         tc.tile_pool(name="psum_ta", bufs=2, space="PSUM") as ppta:
        xr = x_own.rearrange("(n p) e -> p n e", p=128)
        for n in range(0, TN, 2):
            nc.sync.dma_start(out=x_sb[:, n:n + 2, :], in_=xr[:, n:n + 2, :])
        xnT_sb = pa.tile([128, EK, T], BF16)
        for n in range(TN):
            mv, rstd = _ln_stats(nc, pools, x_sb[:, n, :], eps_sb)
            xn_b = ptr.tile([128, E], BF16, tag="xnb")
            nc.gpsimd.tensor_scalar(
                out=xn_b, in0=x_sb[:, n, :], scalar1=mv[:, 0:1], scalar2=rstd,
                op0=ALU.subtract, op1=ALU.mult)
            tp = ppta.tile([128, E], BF16, tag="tp")
            for e in range(EK):
                nc.tensor.transpose(tp[:, e * 128:(e + 1) * 128],
                                    xn_b[:, e * 128:(e + 1) * 128], identity)
            nc.vector.tensor_copy(xnT_sb[:, :, n * 128:(n + 1) * 128],
                                  tp.rearrange("p (e t) -> p e t", e=EK))
        if DEBUG_OUTPUTS:
            nc.sync.dma_start(out=io["dbg_xnT"],
                              in_=xnT_sb.rearrange("p k t -> p (k t)"))

        # ================= Phase B: K, V, Q projections =================
        with tc.tile_pool(name="wproj", bufs=2) as pw, \
             tc.tile_pool(name="psum_b", bufs=3, space="PSUM") as pps:
            # K^T first (gates the AllGather), then V, local q^T last.
            wk_sb = pw.tile([128, EK, E], BF16, tag="w")
            wkr = wkT.rearrange("(k p) f -> p k f", p=128)
            for k in range(EK):
                nc.sync.dma_start(out=wk_sb[:, k, :], in_=wkr[:, k, :])
            for m in range(EK):
                ps = pps.tile([128, T], F32, tag="mm")
                for k in range(EK):
                    nc.tensor.matmul(ps, wk_sb[:, k, m * 128:(m + 1) * 128],
                                     xnT_sb[:, k, :], start=(k == 0),
                                     stop=(k == EK - 1))
                kcp = ptr.tile([128, T], BF16, tag="kcp")
                nc.scalar.add(out=kcp, in_=ps, add=bkT_sb[:, m:m + 1])
                nc.sync.dma_start(out=ag_k_in[m * 128:(m + 1) * 128, :], in_=kcp)

            # V: token-major [T, 768] (v bias folded into bo host-side)
            wv_sb = pw.tile([128, EK, E], BF16, tag="w")
            wvr = wvT.rearrange("(k p) f -> p k f", p=128)
            for k in range(EK):
                nc.sync.dma_start(out=wv_sb[:, k, :], in_=wvr[:, k, :])
            for n in range(TN):
                ps1 = pps.tile([128, 512], F32, tag="mm")
                ps2 = pps.tile([128, 256], F32, tag="mm2")
                for k in range(EK):
                    lhsT = xnT_sb[:, k, n * 128:(n + 1) * 128]
                    nc.tensor.matmul(ps1, lhsT, wv_sb[:, k, 0:512],
                                     start=(k == 0), stop=(k == EK - 1))
                    nc.tensor.matmul(ps2, lhsT, wv_sb[:, k, 512:768],
                                     start=(k == 0), stop=(k == EK - 1))
                vcp = ptr.tile([128, E], BF16, tag="vcp")
                nc.vector.tensor_copy(vcp[:, 0:512], ps1)
                nc.vector.tensor_copy(vcp[:, 512:768], ps2)
                nc.sync.dma_start(
                    out=ag_v_in.rearrange("(n p) e -> p n e", p=128)[:, n, :],
                    in_=vcp)

            # q^T: local only, overlaps the in-flight AllGathers
            wq_sb = pw.tile([128, EK, E], BF16, tag="w")
            wqr = wqT.rearrange("(k p) f -> p k f", p=128)
            for k in range(EK):
                nc.sync.dma_start(out=wq_sb[:, k, :], in_=wqr[:, k, :])
            for m in range(EK):
                ps = pps.tile([128, T], F32, tag="mm")
                for k in range(EK):
                    nc.tensor.matmul(ps, wq_sb[:, k, m * 128:(m + 1) * 128],
                                     xnT_sb[:, k, :], start=(k == 0),
                                     stop=(k == EK - 1))
                nc.scalar.add(out=qT_sb[:, m, :], in_=ps, add=bqT_sb[:, m:m + 1])

    if DEBUG_OUTPUTS:
        nc.sync.dma_start(out=io["dbg_qT"], in_=qT_sb.rearrange("p k t -> p (k t)"))

    # ================= Phase C: AllGather K^T and V =================
    if not skip_collectives:
        nc.gpsimd.collective_compute("AllGather", ALU.bypass,
                                     replica_groups=groups,
                                     ins=[ag_k_in[:]], outs=[ag_k_out[:]])
        nc.gpsimd.collective_compute("AllGather", ALU.bypass,
                                     replica_groups=groups,
                                     ins=[ag_v_in[:]], outs=[ag_v_out[:]])

    # ---- open late-phase pools early so weight DMAs overlap attention ----
    span_fgh = open_pool("span_fgh", bufs=1)
    hT_sb = span_fgh.tile([128, FK, T], BF16)     # FFN hidden, feature-major
    w1_sb = span_fgh.tile([128, EK, FF], BF16)
    w1r = w1T.rearrange("(k p) f -> p k f", p=128)
    for k in range(EK):
        for j in range(2):
            nc.sync.dma_start(out=w1_sb[:, k, j * 1536:(j + 1) * 1536],
                              in_=w1r[:, k, j * 1536:(j + 1) * 1536])
    w2_sb = span_fgh.tile([128, FK, E], BF16)
    w2r = w2T.rearrange("(k p) f -> p k f", p=128)
    for k in range(0, FK, 2):
        nc.sync.dma_start(out=w2_sb[:, k:k + 2, :], in_=w2r[:, k:k + 2, :])

    # bo pre-add into x (Pool, overlaps attention): res = x + bo + attn@woT
    for n in range(TN):
        nc.gpsimd.tensor_add(x_sb[:, n, :], x_sb[:, n, :], bo_rep)

    # ================= Phase D: attention =================
    kgr = ag_k_out.rearrange("(r hp p) t -> p r hp t", r=GROUP, hp=EK, p=128)
    vgr = ag_v_out.rearrange("(t p) e -> p t e", p=128)

    # zero-fill the padded prefix of each ex_all region (exp writes only the
    # suffix [s*128:T]; PV reads the full 512 cols)
    for s in range(1, TN):
        for r in range(GROUP):
            nc.gpsimd.memset(ex_all[:, s * GROUP + r, 0:s * 128], 0.0)

    with tc.tile_pool(name="attn_kv", bufs=2) as pkv, \
         tc.tile_pool(name="attn_v", bufs=3) as pv, \
         tc.tile_pool(name="attn_r", bufs=2) as pr, \
         tc.tile_pool(name="psum_s", bufs=2, space="PSUM") as pps_s, \
         tc.tile_pool(name="psum_a", bufs=2, space="PSUM") as pps_a:
        for h in range(KHEADS):
            hp, ho = h // 2, (h % 2) * 64
            if h % 2 == 0:
                khead2 = pkv.tile([128, GROUP, T], BF16, tag="khead")
                nc.sync.dma_start(out=khead2, in_=kgr[:, :, hp, :])
            vhead = pv.tile([128, SB, D + 1], BF16, tag="vhead")
            nc.sync.dma_start(out=vhead[:, :, 0:D],
                              in_=vgr[:, :, h * 64:(h + 1) * 64])
            nc.vector.memset(vhead[:, :, D:D + 1], 1.0)

            ps_attn = pps_a.tile([D + 1, T], F32, tag="pattn")
            qh = qT_sb[ho:ho + 64, hp, :]

            def strided(tile2d, col, pitch, cnt, w):
                base = tile2d[:, col:col + w]
                return bass.AP(tensor=base.tensor, offset=base.offset,
                               ap=[list(base.ap[0]), [pitch, cnt], [1, w]])

            for s, rs, offs in CHUNKS3:
                W = (TN - s) * 128
                n = len(rs)
                idx0 = s * GROUP + rs[0]
                psc = pps_s.tile([128, 1024], F32, tag="psc")
                for i, r in enumerate(rs):
                    nc.tensor.matmul(psc[:, offs[i]:offs[i] + W],
                                     khead2[ho:ho + 64, r, s * 128:(s + 1) * 128],
                                     qh[:, s * 128:T],
                                     start=True, stop=True)
                src = (psc[:, 0:W] if n == 1
                       else strided(psc, 0, offs[1] - offs[0], n, W))
                ex_out = ex_all[:, idx0:idx0 + n, s * 128:T]
                nc.scalar.activation(out=ex_out, in_=src,
                                     func=AF.Exp, scale=SCALE)
                # multiplicative 0/1 causal mask on the diagonal-slot blocks
                # (Pool engine, SBUF-only, off the PE critical path)
                if not KSKIP_MASK:
                    dex = ex_all[:, idx0:idx0 + n, s * 128:(s + 1) * 128]
                    nc.gpsimd.tensor_mul(dex, dex, mask01[:, rs[0]:rs[0] + n, :])

            if KPV40:
                for s in range(TN):
                    for r in range(GROUP):
                        for qs in range(s, TN):
                            nc.tensor.matmul(
                                ps_attn[:, qs * 128:(qs + 1) * 128],
                                vhead[:, r * TN + s, :],
                                ex_all[:, s * GROUP + r, qs * 128:(qs + 1) * 128],
                                start=(s == 0 and r == 0),
                                stop=(s == qs and r == GROUP - 1))
            else:
                for s in range(TN):
                    for r in range(GROUP):
                        nc.tensor.matmul(ps_attn, vhead[:, r * TN + s, :],
                                         ex_all[:, s * GROUP + r, :],
                                         start=(s == 0 and r == 0),
                                         stop=(s == TN - 1 and r == GROUP - 1))

            # normalize: attnT[d, t] = ps_attn[d, t] / ps_attn[64, t]
            r_row = pr.tile([1, T], F32, tag="rrow")
            nc.vector.reciprocal(out=r_row, in_=ps_attn[D:D + 1, :])
            r64 = pr.tile([D, T], F32, tag="r64")
            nc.gpsimd.partition_broadcast(r64, r_row)
            nc.vector.tensor_mul(attnT_sb[ho:ho + 64, hp, :],
                                 ps_attn[0:D, :], r64)

    if DEBUG_OUTPUTS:
        nc.sync.dma_start(out=io["dbg_attnT"],
                          in_=attnT_sb.rearrange("p k t -> p (k t)"))

    # ================= Phase E: out-proj + residual + LN2 =================
    span_fg = open_pool("span_fg", bufs=1)
    y2nT_sb = span_fg.tile([128, EK, T], BF16)    # LN2 output, feature-major
    with tc.tile_pool(name="span_e", bufs=1) as pf, \
         tc.tile_pool(name="tr_e", bufs=2) as ptrf, \
         tc.tile_pool(name="psum_te", bufs=2, space="PSUM") as pptf, \
         tc.tile_pool(name="psum_e", bufs=3, space="PSUM") as ppsf:
        wo_sb = pf.tile([128, EK, E], BF16)
        wor = woT.rearrange("(k p) f -> p k f", p=128)
        for k in range(EK):
            nc.sync.dma_start(out=wo_sb[:, k, :], in_=wor[:, k, :])
        for n in range(TN):
            ps1 = ppsf.tile([128, 512], F32, tag="mm")
            ps2 = ppsf.tile([128, 256], F32, tag="mm2")
            for k in range(EK):
                lhsT = attnT_sb[:, k, n * 128:(n + 1) * 128]
                nc.tensor.matmul(ps1, lhsT, wo_sb[:, k, 0:512],
                                 start=(k == 0), stop=(k == EK - 1))
                nc.tensor.matmul(ps2, lhsT, wo_sb[:, k, 512:768],
                                 start=(k == 0), stop=(k == EK - 1))
            nc.vector.tensor_add(res_sb[:, n, 0:512], ps1, x_sb[:, n, 0:512])
            nc.vector.tensor_add(res_sb[:, n, 512:768], ps2, x_sb[:, n, 512:768])
            mv, rstd = _ln_stats(nc, pools, res_sb[:, n, :], eps_sb)
            y2n_b = ptrf.tile([128, E], BF16, tag="y2nb")
            nc.gpsimd.tensor_scalar(
                out=y2n_b, in0=res_sb[:, n, :], scalar1=mv[:, 0:1], scalar2=rstd,
                op0=ALU.subtract, op1=ALU.mult)
            tp = pptf.tile([128, E], BF16, tag="tp")
            for e in range(EK):
                nc.tensor.transpose(tp[:, e * 128:(e + 1) * 128],
                                    y2n_b[:, e * 128:(e + 1) * 128], identity)
            nc.vector.tensor_copy(y2nT_sb[:, :, n * 128:(n + 1) * 128],
                                  tp.rearrange("p (e t) -> p e t", e=EK))
            # pre-add b2 into res (Pool) for the final residual
            nc.gpsimd.tensor_add(res_sb[:, n, :], res_sb[:, n, :], b2_rep)

    if DEBUG_OUTPUTS:
        nc.sync.dma_start(out=io["dbg_res"], in_=res_sb.rearrange("p n e -> p (n e)"))

    # ================= Phase F: FFN1 + GELU -> hT =================
    with tc.tile_pool(name="psum_f", bufs=4, space="PSUM") as ppsg:
        for m in range(FK if not KSKIP_FFN else 2):
            ps = ppsg.tile([128, T], F32, tag="mm")
            kf1 = EK if not KSKIP_FFN else 1
            for k in range(kf1):
                nc.tensor.matmul(ps, w1_sb[:, k, m * 128:(m + 1) * 128],
                                 y2nT_sb[:, k, :], start=(k == 0),
                                 stop=(k == kf1 - 1))
            nc.scalar.activation(out=hT_sb[:, m, :], in_=ps, func=AF.Gelu,
                                 bias=b1_sb[:, m:m + 1], scale=1.0)

    if DEBUG_OUTPUTS:
        nc.sync.dma_start(out=io["dbg_hT"], in_=hT_sb.rearrange("p k t -> p (k t)"))

    # close span_fg (y2nT dead)
    ctx_pools.remove(span_fg._cm)
    span_fg._cm.__exit__(None, None, None)

    # ================= Phase G: FFN2 + residual -> out =================
    with tc.tile_pool(name="psum_g", bufs=3, space="PSUM") as ppsh:
        for n in range(TN):
            ps1 = ppsh.tile([128, 512], F32, tag="mm")
            ps2 = ppsh.tile([128, 256], F32, tag="mm2")
            kf2 = FK if not KSKIP_FFN else 2
            for k in range(kf2):
                lhsT = hT_sb[:, k, n * 128:(n + 1) * 128]
                nc.tensor.matmul(ps1, lhsT, w2_sb[:, k, 0:512],
                                 start=(k == 0), stop=(k == kf2 - 1))
                nc.tensor.matmul(ps2, lhsT, w2_sb[:, k, 512:768],
                                 start=(k == 0), stop=(k == kf2 - 1))
            nc.vector.tensor_add(res_sb[:, n, 0:512], res_sb[:, n, 0:512], ps1)
            nc.vector.tensor_add(res_sb[:, n, 512:768], res_sb[:, n, 512:768], ps2)
        outr = out.rearrange("(n p) e -> p n e", p=128)
        for n in range(0, TN, 2):
            nc.sync.dma_start(out=outr[:, n:n + 2, :], in_=res_sb[:, n:n + 2, :])

    for p in reversed(ctx_pools):
        p.__exit__(None, None, None)


_CACHED = {}


def _get_module(repeat=1, loop_n=None, skip_collectives=False):
    key = ("nc", repeat, loop_n, skip_collectives)
    if key not in _CACHED:
        nc = bacc.Bacc("TRN2", target_bir_lowering=False, debug=False,
                       enable_asserts=False, num_devices=NCORES)
        io = declare_io(nc)
        with tile.TileContext(nc) as tc:
            if loop_n is not None:
                with tc.For_i(0, loop_n, 1):
                    build_kernel_body(tc, io, skip_collectives=True)
            else:
                for _ in range(repeat):
                    build_kernel_body(tc, io, skip_collectives=skip_collectives)
        nc.compile()
        _CACHED[key] = nc
    return _CACHED[key]


def make_in_maps(x, wq, wk, wv, wo, bo, w1, b1, w2, b2, gamma, beta):
    import ml_dtypes
    f = np.float32
    bf = ml_dtypes.bfloat16
    xf = np.asarray(x, f)
    wq_, wk_, wv_, wo_ = (np.asarray(w, f) for w in (wq, wk, wv, wo))
    w1_, w2_ = np.asarray(w1, f), np.asarray(w2, f)
    gam, bet = np.asarray(gamma, f), np.asarray(beta, f)
    bo_, b1_, b2_ = np.asarray(bo, f), np.asarray(b1, f), np.asarray(b2, f)

    # gamma folds into weight columns; beta becomes per-output biases
    bq = wq_ @ bet
    bk = wk_ @ bet
    bv = wv_ @ bet
    bo_eff = bo_ + wo_ @ bv
    b1_eff = b1_ + w1_ @ bet

    common = {
        "wqT": np.ascontiguousarray((wq_.T * gam[:, None]).astype(bf)),
        "wkT": np.ascontiguousarray((wk_.T * gam[:, None]).astype(bf)),
        "wvT": np.ascontiguousarray((wv_.T * gam[:, None]).astype(bf)),
        "woT": np.ascontiguousarray(wo_.T.astype(bf)),
        "w1T": np.ascontiguousarray((w1_.T * gam[:, None]).astype(bf)),
        "w2T": np.ascontiguousarray(w2_.T.astype(bf)),
        "b1rs": np.ascontiguousarray(b1_eff.reshape(FK, 128).T),
        "bo_row": bo_eff.reshape(1, E),
        "b2_row": b2_.reshape(1, E),
        "bqT": np.ascontiguousarray(bq.reshape(EK, 128).T),
        "bkT": np.ascontiguousarray(bk.reshape(EK, 128).T),
    }
    # multiplicative diagonal-slot masks [k, q]:
    # r<j -> 1, r==j -> tri(k<=q), r>j -> 0
    kidx = np.arange(128)[:, None]
    qidx = np.arange(128)[None, :]
    tri = (kidx <= qidx).astype(f)
    in_maps = []
    for c in range(NCORES):
        b, j = c // GROUP, c % GROUP
        tiles = [j, j + 4, j + 8, j + 12]
        m = dict(common)
        m["x_own"] = np.ascontiguousarray(
            np.concatenate([xf[b, t * 128:(t + 1) * 128, :] for t in tiles]))
        mk = np.zeros((GROUP, 128, 128), f)
        mk[0:j] = 1.0
        mk[j] = tri
        m["masks"] = mk.astype(bf)
        in_maps.append(m)
    return in_maps


def run(in_maps, trace=False):
    nc = _get_module()
    return run_bass_kernel_spmd(nc, in_maps, core_ids=list(range(NCORES)),
                                trace=trace)


def kernel(x, wq, wk, wv, wo, bo, w1, b1, w2, b2, gamma, beta):
    in_maps = make_in_maps(x, wq, wk, wv, wo, bo, w1, b1, w2, b2, gamma, beta)
    res = run(in_maps)
    out = np.zeros((B, S, E), np.float32)
    for c in range(NCORES):
        b, j = c // GROUP, c % GROUP
        chunk = res.results[c]["out"]
        for slot, t in enumerate([j, j + 4, j + 8, j + 12]):
            out[b, t * 128:(t + 1) * 128] = chunk[slot * 128:(slot + 1) * 128]
    return out


# revision 44
# speedup vs baseline: 2.5564x; 1.4809x over previous
"""Trainium2 Bass kernel for a dense transformer block (pre-LN MHA + FFN).

Reference computation (B=2, S=2048, E=768, H=12, D=64, FF=3072):
    res = x
    xn = LN(x, gamma, beta)
    q,k,v = xn @ wq.T, xn @ wk.T, xn @ wv.T          (per-head D=64)
    attn = causal_softmax(q k^T / sqrt(D)) v
    res = res + attn @ wo.T + bo
    y = LN(res, gamma, beta)
    out = res + gelu(y @ w1.T + b1) @ w2.T + b2

Sharding: 8 cores, token-parallel with BLOCK-INTERLEAVED causal balancing.
Cores 0-3 = batch 0, cores 4-7 = batch 1. Within a batch group, core j owns
query tiles {j, j+4, j+8, j+12} (of 16 tiles x 128 tokens), stored sorted.
K^T and V are AllGather'ed within the group (owner-major layout, so gathered
key tile (owner r, slot s) = global tile 4s+r).

Attention is causal-skipped with a core-UNIFORM program: query slot qs only
visits gathered key slots s <= qs (40 of 64 block-visits). Diagonal-slot
(s == qs) visits get an additive mask (0 / upper-tri -30000 / all -30000,
selected per-core by an input tensor) applied as a PE accumulate-matmul
before exp, so softmax is exact and no DVE mask multiply is needed.

gamma is folded into wq/wk/wv/w1 host-side; beta becomes per-feature biases
(applied on the ACT engine during Q/K PSUM->SBUF copies; V's bias folds into
bo, w1's into b1). The LN kernel is therefore pure normalize.

Engine split: PE matmuls/transposes/mask-adds, ACT exp/GELU/Q-K-bias copies,
DVE stats/reciprocals/residual adds/copies, Pool(gpsimd) LN-apply and
SBUF-side bias adds.
"""

import numpy as np

import concourse.bass as bass
import concourse.mybir as mybir
import concourse.tile as tile
from concourse import bacc
from concourse.bass_utils import run_bass_kernel_spmd
from concourse.masks import make_identity

F32 = mybir.dt.float32
F32R = mybir.dt.float32r
BF16 = mybir.dt.bfloat16
AF = mybir.ActivationFunctionType
ALU = mybir.AluOpType

DEBUG_OUTPUTS = False

import os
KHEADS = int(os.environ.get("KHEADS", "12"))        # timing experiments only
KSKIP_FFN = os.environ.get("KSKIP_FFN", "0") == "1"
KSKIP_MASK = os.environ.get("KSKIP_MASK", "0") == "1"
KPV40 = os.environ.get("KPV40", "0") == "1"         # per-qs-block PV chains

B, S, E, H, FF = 2, 2048, 768, 12, 3072
D = E // H                      # 64
NCORES = 8
T = B * S // NCORES             # 512 tokens per core
TN = T // 128                   # 4 token tiles per core
EK = E // 128                   # 6 feature chunks
FK = FF // 128                  # 24 hidden chunks
GROUP = NCORES // B             # 4 cores per batch
SB = S // 128                   # 16 key tiles per batch
EPS = 1e-5
SCALE = 1.0 / np.sqrt(D)
NEG = -30000.0

# s-major score groups: for gathered key slot s, query slots qs>=s form a
# contiguous suffix of width W=(TN-s)*128. Chunk regions are placed at
# bank-aligned-or-within offsets so no matmul output crosses a PSUM bank.
# (s, owners, psc region offsets)
CHUNKS3 = [
    (0, [0, 1], [0, 512]),
    (0, [2, 3], [0, 512]),
    (1, [0, 1], [0, 512]),
    (1, [2, 3], [0, 512]),
    (2, [0, 1, 2, 3], [0, 256, 512, 768]),
    (3, [0, 1, 2, 3], [0, 128, 256, 384]),
]


def _ln_stats(nc, pools, x_ap, eps_sb):
    """mean/rstd of a [128, 768] tile (free-axis LN). Returns (mv, rstd)."""
    stats = pools["stats"].tile([128, 3, 6], F32, tag="stats")
    mv = pools["stats"].tile([128, 2], F32, tag="mv")
    for g in range(3):
        nc.vector.bn_stats(out=stats[:, g, :], in_=x_ap[:, g * 256:(g + 1) * 256])
    nc.vector.bn_aggr(out=mv, in_=stats)
    rstd = pools["stats"].tile([128, 1], F32, tag="rstd")
    nc.scalar.activation(out=rstd, in_=mv[:, 1:2], func=AF.Sqrt, bias=eps_sb, scale=1.0)
    nc.vector.reciprocal(out=rstd, in_=rstd)
    return mv, rstd


def declare_io(nc):
    io = {}
    io["x_own"] = nc.dram_tensor("x_own", [T, E], F32, kind="ExternalInput").ap()
    for nm in ("wqT", "wkT", "wvT", "woT"):
        io[nm] = nc.dram_tensor(nm, [E, E], BF16, kind="ExternalInput").ap()
    io["w1T"] = nc.dram_tensor("w1T", [E, FF], BF16, kind="ExternalInput").ap()
    io["w2T"] = nc.dram_tensor("w2T", [FF, E], BF16, kind="ExternalInput").ap()
    io["b1rs"] = nc.dram_tensor("b1rs", [128, FK], F32, kind="ExternalInput").ap()
    io["bo_row"] = nc.dram_tensor("bo_row", [1, E], F32, kind="ExternalInput").ap()
    io["b2_row"] = nc.dram_tensor("b2_row", [1, E], F32, kind="ExternalInput").ap()
    io["bqT"] = nc.dram_tensor("bqT", [128, EK], F32, kind="ExternalInput").ap()
    io["bkT"] = nc.dram_tensor("bkT", [128, EK], F32, kind="ExternalInput").ap()
    io["masks"] = nc.dram_tensor("masks", [GROUP, 128, 128], BF16,
                                 kind="ExternalInput").ap()
    io["out"] = nc.dram_tensor("out", [T, E], F32, kind="ExternalOutput").ap()
    if DEBUG_OUTPUTS:
        for nm, shp, dt in (("dbg_xnT", [128, EK * T], BF16),
                            ("dbg_qT", [128, EK * T], BF16),
                            ("dbg_attnT", [128, EK * T], BF16),
                            ("dbg_res", [128, TN * E], F32),
                            ("dbg_hT", [128, FK * T], BF16)):
            io[nm] = nc.dram_tensor(nm, shp, dt, kind="ExternalOutput").ap()
    return io


def build_kernel_body(tc, io, skip_collectives=False):
    nc = tc.nc
    x_own, wqT, wkT, wvT, woT = (io[k] for k in ("x_own", "wqT", "wkT", "wvT", "woT"))
    w1T, w2T, b1rs = io["w1T"], io["w2T"], io["b1rs"]
    bo_row, b2_row = io["bo_row"], io["b2_row"]
    bqT_in, bkT_in, masks, out = io["bqT"], io["bkT"], io["masks"], io["out"]

    pools = {}
    ctx_pools = []

    def open_pool(name, **kw):
        cm = tc.tile_pool(name=name, **kw)
        pool = cm.__enter__()
        ctx_pools.append(cm)
        pool._cm = cm
        return pool

    persist = open_pool("persist", bufs=1)
    pools["stats"] = open_pool("stats", bufs=3)
    dram = open_pool("dram", bufs=1, space="DRAM")

    # ---- constants ----
    identity = persist.tile([128, 128], BF16)
    make_identity(nc, identity)

    ones_all = persist.tile([128, 128], F32)
    nc.vector.memset(ones_all, 1.0)

    eps_sb = persist.tile([128, 1], F32)
    nc.vector.memset(eps_sb, EPS)

    def rep128(name, row_ap):
        t = persist.tile([128, E], F32, name=name)
        src = bass.AP(tensor=row_ap.tensor, offset=row_ap.offset,
                      ap=[[0, 128]] + list(row_ap.ap[1:]))
        nc.sync.dma_start(out=t, in_=src)
        return t

    bo_rep = rep128("bo_rep", bo_row)
    b2_rep = rep128("b2_rep", b2_row)
    b1_sb = persist.tile([128, FK], F32)
    nc.sync.dma_start(out=b1_sb, in_=b1rs)
    bqT_sb = persist.tile([128, EK], F32)
    nc.sync.dma_start(out=bqT_sb, in_=bqT_in)
    bkT_sb = persist.tile([128, EK], F32)
    nc.sync.dma_start(out=bkT_sb, in_=bkT_in)
    mask01 = persist.tile([128, GROUP, 128], BF16)
    nc.sync.dma_start(out=mask01, in_=masks.rearrange("r p q -> p r q"))

    # ---- long-lived activations ----
    qT_sb = persist.tile([128, EK, T], BF16)      # q, feature-major, 2-head packed
    attnT_sb = persist.tile([128, EK, T], BF16)   # attention out, feature-major
    res_sb = persist.tile([128, TN, E], F32)      # post-attn residual, token-major
    x_sb = persist.tile([128, TN, E], F32)        # input x (token-major), reused
    # exp'd scores, region idx = s*GROUP+r holds [zeros(s*128) | exp suffix]
    ex_all = persist.tile([128, SB, T], BF16)

    # ---- AllGather bounce buffers ----
    ag_k_in = dram.tile([E, T], BF16)
    ag_v_in = dram.tile([T, E], BF16)
    ag_k_out = dram.tile([GROUP * E, T], BF16)
    ag_v_out = dram.tile([S, E], BF16)
    groups = [list(range(GROUP)), list(range(GROUP, NCORES))]

    # ================= Phase A: load x, LN1, transpose =================
    with tc.tile_pool(name="span_a", bufs=1) as pa, \
         tc.tile_pool(name="tr_a", bufs=2) as ptr, \
         tc.tile_pool(name="psum_ta", bufs=2, space="PSUM") as ppta:
        xr = x_own.rearrange("(n p) e -> p n e", p=128)
        for n in range(0, TN, 2):
            nc.sync.dma_start(out=x_sb[:, n:n + 2, :], in_=xr[:, n:n + 2, :])
        xnT_sb = pa.tile([128, EK, T], BF16)
        for n in range(TN):
            mv, rstd = _ln_stats(nc, pools, x_sb[:, n, :], eps_sb)
            xn_b = ptr.tile([128, E], BF16, tag="xnb")
            nc.vector.tensor_scalar(
                out=xn_b, in0=x_sb[:, n, :], scalar1=mv[:, 0:1], scalar2=rstd,
                op0=ALU.subtract, op1=ALU.mult)
            tp = ppta.tile([128, E], BF16, tag="tp")
            for e in range(EK):
                nc.tensor.transpose(tp[:, e * 128:(e + 1) * 128],
                                    xn_b[:, e * 128:(e + 1) * 128], identity)
            nc.vector.tensor_copy(xnT_sb[:, :, n * 128:(n + 1) * 128],
                                  tp.rearrange("p (e t) -> p e t", e=EK))
        if DEBUG_OUTPUTS:
            nc.sync.dma_start(out=io["dbg_xnT"],
                              in_=xnT_sb.rearrange("p k t -> p (k t)"))

        # ================= Phase B: K, V, Q projections =================
        with tc.tile_pool(name="wproj", bufs=2) as pw, \
             tc.tile_pool(name="psum_b", bufs=3, space="PSUM") as pps:
            # K^T first (gates the AllGather), then V, local q^T last.
            wk_sb = pw.tile([128, EK, E], BF16, tag="w")
            wkr = wkT.rearrange("(k p) f -> p k f", p=128)
            for k in range(EK):
                nc.sync.dma_start(out=wk_sb[:, k, :], in_=wkr[:, k, :])
            for m in range(EK):
                ps = pps.tile([128, T], F32, tag="mm")
                for k in range(EK):
                    nc.tensor.matmul(ps, wk_sb[:, k, m * 128:(m + 1) * 128],
                                     xnT_sb[:, k, :], start=(k == 0),
                                     stop=(k == EK - 1))
                kcp = ptr.tile([128, T], BF16, tag="kcp")
                nc.scalar.add(out=kcp, in_=ps, add=bkT_sb[:, m:m + 1])
                nc.sync.dma_start(out=ag_k_in[m * 128:(m + 1) * 128, :], in_=kcp)

            # V: token-major [T, 768] (v bias folded into bo host-side)
            wv_sb = pw.tile([128, EK, E], BF16, tag="w")
            wvr = wvT.rearrange("(k p) f -> p k f", p=128)
            for k in range(EK):
                nc.sync.dma_start(out=wv_sb[:, k, :], in_=wvr[:, k, :])
            for n in range(TN):
                ps1 = pps.tile([128, 512], F32, tag="mm")
                ps2 = pps.tile([128, 256], F32, tag="mm2")
                for k in range(EK):
                    lhsT = xnT_sb[:, k, n * 128:(n + 1) * 128]
                    nc.tensor.matmul(ps1, lhsT, wv_sb[:, k, 0:512],
                                     start=(k == 0), stop=(k == EK - 1))
                    nc.tensor.matmul(ps2, lhsT, wv_sb[:, k, 512:768],
                                     start=(k == 0), stop=(k == EK - 1))
                vcp = ptr.tile([128, E], BF16, tag="vcp")
                nc.vector.tensor_copy(vcp[:, 0:512], ps1)
                nc.vector.tensor_copy(vcp[:, 512:768], ps2)
                nc.sync.dma_start(
                    out=ag_v_in.rearrange("(n p) e -> p n e", p=128)[:, n, :],
                    in_=vcp)

            # q^T: local only, overlaps the in-flight AllGathers
            wq_sb = pw.tile([128, EK, E], BF16, tag="w")
            wqr = wqT.rearrange("(k p) f -> p k f", p=128)
            for k in range(EK):
                nc.sync.dma_start(out=wq_sb[:, k, :], in_=wqr[:, k, :])
            for m in range(EK):
                ps = pps.tile([128, T], F32, tag="mm")
                for k in range(EK):
                    nc.tensor.matmul(ps, wq_sb[:, k, m * 128:(m + 1) * 128],
                                     xnT_sb[:, k, :], start=(k == 0),
                                     stop=(k == EK - 1))
                nc.scalar.add(out=qT_sb[:, m, :], in_=ps, add=bqT_sb[:, m:m + 1])

    if DEBUG_OUTPUTS:
        nc.sync.dma_start(out=io["dbg_qT"], in_=qT_sb.rearrange("p k t -> p (k t)"))

    # ================= Phase C: AllGather K^T and V =================
    if not skip_collectives:
        nc.gpsimd.collective_compute("AllGather", ALU.bypass,
                                     replica_groups=groups,
                                     ins=[ag_k_in[:]], outs=[ag_k_out[:]])
        nc.gpsimd.collective_compute("AllGather", ALU.bypass,
                                     replica_groups=groups,
                                     ins=[ag_v_in[:]], outs=[ag_v_out[:]])

    # ---- open late-phase pools early so weight DMAs overlap attention ----
    span_fgh = open_pool("span_fgh", bufs=1)
    hT_sb = span_fgh.tile([128, FK, T], BF16)     # FFN hidden, feature-major
    w1_sb = span_fgh.tile([128, EK, FF], BF16)
    w1r = w1T.rearrange("(k p) f -> p k f", p=128)
    for k in range(EK):
        for j in range(2):
            nc.sync.dma_start(out=w1_sb[:, k, j * 1536:(j + 1) * 1536],
                              in_=w1r[:, k, j * 1536:(j + 1) * 1536])
    w2_sb = span_fgh.tile([128, FK, E], BF16)
    w2r = w2T.rearrange("(k p) f -> p k f", p=128)
    for k in range(0, FK, 2):
        nc.sync.dma_start(out=w2_sb[:, k:k + 2, :], in_=w2r[:, k:k + 2, :])

    # bo pre-add into x (Pool, overlaps attention): res = x + bo + attn@woT
    for n in range(TN):
        nc.gpsimd.tensor_add(x_sb[:, n, :], x_sb[:, n, :], bo_rep)

    # ================= Phase D: attention =================
    kgr = ag_k_out.rearrange("(r hp p) t -> p r hp t", r=GROUP, hp=EK, p=128)
    vgr = ag_v_out.rearrange("(t p) e -> p t e", p=128)

    # zero-fill the padded prefix of each ex_all region (exp writes only the
    # suffix [s*128:T]; PV reads the full 512 cols)
    for s in range(1, TN):
        for r in range(GROUP):
            nc.vector.memset(ex_all[:, s * GROUP + r, 0:s * 128], 0.0)

    with tc.tile_pool(name="attn_kv", bufs=2) as pkv, \
         tc.tile_pool(name="attn_v", bufs=3) as pv, \
         tc.tile_pool(name="attn_r", bufs=2) as pr, \
         tc.tile_pool(name="attn_rd", bufs=2, space="DRAM") as pdram, \
         tc.tile_pool(name="psum_s", bufs=2, space="PSUM") as pps_s, \
         tc.tile_pool(name="psum_a", bufs=2, space="PSUM") as pps_a:
        for h in range(KHEADS):
            hp, ho = h // 2, (h % 2) * 64
            if h % 2 == 0:
                khead2 = pkv.tile([128, GROUP, T], BF16, tag="khead")
                nc.sync.dma_start(out=khead2, in_=kgr[:, :, hp, :])
            vhead = pv.tile([128, SB, D + 1], BF16, tag="vhead")
            nc.sync.dma_start(out=vhead[:, :, 0:D],
                              in_=vgr[:, :, h * 64:(h + 1) * 64])
            nc.vector.memset(vhead[:, :, D:D + 1], 1.0)

            ps_attn = pps_a.tile([D + 1, T], F32, tag="pattn")
            qh = qT_sb[ho:ho + 64, hp, :]

            def strided(tile2d, col, pitch, cnt, w):
                base = tile2d[:, col:col + w]
                return bass.AP(tensor=base.tensor, offset=base.offset,
                               ap=[list(base.ap[0]), [pitch, cnt], [1, w]])

            for s, rs, offs in CHUNKS3:
                W = (TN - s) * 128
                n = len(rs)
                idx0 = s * GROUP + rs[0]
                psc = pps_s.tile([128, 1024], F32, tag="psc")
                for i, r in enumerate(rs):
                    nc.tensor.matmul(psc[:, offs[i]:offs[i] + W],
                                     khead2[ho:ho + 64, r, s * 128:(s + 1) * 128],
                                     qh[:, s * 128:T],
                                     start=True, stop=True)
                src = (psc[:, 0:W] if n == 1
                       else strided(psc, 0, offs[1] - offs[0], n, W))
                ex_out = ex_all[:, idx0:idx0 + n, s * 128:T]
                nc.scalar.activation(out=ex_out, in_=src,
                                     func=AF.Exp, scale=SCALE)
                # multiplicative 0/1 causal mask on the diagonal-slot blocks
                # (Pool engine, SBUF-only, off the PE critical path)
                if not KSKIP_MASK:
                    dex = ex_all[:, idx0:idx0 + n, s * 128:(s + 1) * 128]
                    nc.gpsimd.tensor_mul(dex, dex, mask01[:, rs[0]:rs[0] + n, :])

            if KPV40:
                for s in range(TN):
                    for r in range(GROUP):
                        for qs in range(s, TN):
                            nc.tensor.matmul(
                                ps_attn[:, qs * 128:(qs + 1) * 128],
                                vhead[:, r * TN + s, :],
                                ex_all[:, s * GROUP + r, qs * 128:(qs + 1) * 128],
                                start=(s == 0 and r == 0),
                                stop=(s == qs and r == GROUP - 1))
            else:
                for s in range(TN):
                    for r in range(GROUP):
                        nc.tensor.matmul(ps_attn, vhead[:, r * TN + s, :],
                                         ex_all[:, s * GROUP + r, :],
                                         start=(s == 0 and r == 0),
                                         stop=(s == TN - 1 and r == GROUP - 1))

            # normalize: attnT[d, t] = ps_attn[d, t] / ps_attn[64, t]
            r_row = pr.tile([1, T], F32, tag="rrow")
            nc.vector.reciprocal(out=r_row, in_=ps_attn[D:D + 1, :])
            r64 = pr.tile([D, T], F32, tag="r64")
            r_dram = pdram.tile([1, T], F32, tag="rdram")
            nc.sync.dma_start(out=r_dram, in_=r_row)
            r_src = bass.AP(tensor=r_dram.tensor, offset=r_dram.offset,
                            ap=[[0, D]] + [list(a) for a in r_dram.ap[1:]])
            nc.sync.dma_start(out=r64, in_=r_src)
            nc.vector.tensor_mul(attnT_sb[ho:ho + 64, hp, :],
                                 ps_attn[0:D, :], r64)

    if DEBUG_OUTPUTS:
        nc.sync.dma_start(out=io["dbg_attnT"],
                          in_=attnT_sb.rearrange("p k t -> p (k t)"))

    # ================= Phase E: out-proj + residual + LN2 =================
    span_fg = open_pool("span_fg", bufs=1)
    y2nT_sb = span_fg.tile([128, EK, T], BF16)    # LN2 output, feature-major
    with tc.tile_pool(name="span_e", bufs=1) as pf, \
         tc.tile_pool(name="tr_e", bufs=2) as ptrf, \
         tc.tile_pool(name="psum_te", bufs=2, space="PSUM") as pptf, \
         tc.tile_pool(name="psum_e", bufs=3, space="PSUM") as ppsf:
        wo_sb = pf.tile([128, EK, E], BF16)
        wor = woT.rearrange("(k p) f -> p k f", p=128)
        for k in range(EK):
            nc.sync.dma_start(out=wo_sb[:, k, :], in_=wor[:, k, :])
        for n in range(TN):
            ps1 = ppsf.tile([128, 512], F32, tag="mm")
            ps2 = ppsf.tile([128, 256], F32, tag="mm2")
            for k in range(EK):
                lhsT = attnT_sb[:, k, n * 128:(n + 1) * 128]
                nc.tensor.matmul(ps1, lhsT, wo_sb[:, k, 0:512],
                                 start=(k == 0), stop=(k == EK - 1))
                nc.tensor.matmul(ps2, lhsT, wo_sb[:, k, 512:768],
                                 start=(k == 0), stop=(k == EK - 1))
            nc.vector.tensor_add(res_sb[:, n, 0:512], ps1, x_sb[:, n, 0:512])
            nc.vector.tensor_add(res_sb[:, n, 512:768], ps2, x_sb[:, n, 512:768])
            mv, rstd = _ln_stats(nc, pools, res_sb[:, n, :], eps_sb)
            y2n_b = ptrf.tile([128, E], BF16, tag="y2nb")
            nc.vector.tensor_scalar(
                out=y2n_b, in0=res_sb[:, n, :], scalar1=mv[:, 0:1], scalar2=rstd,
                op0=ALU.subtract, op1=ALU.mult)
            tp = pptf.tile([128, E], BF16, tag="tp")
            for e in range(EK):
                nc.tensor.transpose(tp[:, e * 128:(e + 1) * 128],
                                    y2n_b[:, e * 128:(e + 1) * 128], identity)
            nc.vector.tensor_copy(y2nT_sb[:, :, n * 128:(n + 1) * 128],
                                  tp.rearrange("p (e t) -> p e t", e=EK))
            # pre-add b2 into res (Pool) for the final residual
            nc.gpsimd.tensor_add(res_sb[:, n, :], res_sb[:, n, :], b2_rep)

    if DEBUG_OUTPUTS:
        nc.sync.dma_start(out=io["dbg_res"], in_=res_sb.rearrange("p n e -> p (n e)"))

    # ================= Phase F: FFN1 + GELU -> hT =================
    with tc.tile_pool(name="psum_f", bufs=4, space="PSUM") as ppsg:
        for m in range(FK if not KSKIP_FFN else 2):
            ps = ppsg.tile([128, T], F32, tag="mm")
            kf1 = EK if not KSKIP_FFN else 1
            for k in range(kf1):
                nc.tensor.matmul(ps, w1_sb[:, k, m * 128:(m + 1) * 128],
                                 y2nT_sb[:, k, :], start=(k == 0),
                                 stop=(k == kf1 - 1))
            nc.scalar.activation(out=hT_sb[:, m, :], in_=ps, func=AF.Gelu,
                                 bias=b1_sb[:, m:m + 1], scale=1.0)

    if DEBUG_OUTPUTS:
        nc.sync.dma_start(out=io["dbg_hT"], in_=hT_sb.rearrange("p k t -> p (k t)"))

    # close span_fg (y2nT dead)
    ctx_pools.remove(span_fg._cm)
    span_fg._cm.__exit__(None, None, None)

    # ================= Phase G: FFN2 + residual -> out =================
    with tc.tile_pool(name="psum_g", bufs=3, space="PSUM") as ppsh:
        for n in range(TN):
            ps1 = ppsh.tile([128, 512], F32, tag="mm")
            ps2 = ppsh.tile([128, 256], F32, tag="mm2")
            kf2 = FK if not KSKIP_FFN else 2
            for k in range(kf2):
                lhsT = hT_sb[:, k, n * 128:(n + 1) * 128]
                nc.tensor.matmul(ps1, lhsT, w2_sb[:, k, 0:512],
                                 start=(k == 0), stop=(k == kf2 - 1))
                nc.tensor.matmul(ps2, lhsT, w2_sb[:, k, 512:768],
                                 start=(k == 0), stop=(k == kf2 - 1))
            nc.vector.tensor_add(res_sb[:, n, 0:512], res_sb[:, n, 0:512], ps1)
            nc.vector.tensor_add(res_sb[:, n, 512:768], res_sb[:, n, 512:768], ps2)
        outr = out.rearrange("(n p) e -> p n e", p=128)
        for n in range(0, TN, 2):
            nc.sync.dma_start(out=outr[:, n:n + 2, :], in_=res_sb[:, n:n + 2, :])

    for p in reversed(ctx_pools):
        p.__exit__(None, None, None)


_CACHED = {}


def _get_module(repeat=1, loop_n=None, skip_collectives=False):
    key = ("nc", repeat, loop_n, skip_collectives)
    if key not in _CACHED:
        nc = bacc.Bacc("TRN2", target_bir_lowering=False, debug=False,
                       enable_asserts=False, num_devices=NCORES)
        io = declare_io(nc)
        with tile.TileContext(nc) as tc:
            if loop_n is not None:
                with tc.For_i(0, loop_n, 1):
                    build_kernel_body(tc, io, skip_collectives=True)
            else:
                for _ in range(repeat):
                    build_kernel_body(tc, io, skip_collectives=skip_collectives)
        nc.compile()
        _CACHED[key] = nc
    return _CACHED[key]


def make_in_maps(x, wq, wk, wv, wo, bo, w1, b1, w2, b2, gamma, beta):
    import ml_dtypes
    f = np.float32
    bf = ml_dtypes.bfloat16
    xf = np.asarray(x, f)
    wq_, wk_, wv_, wo_ = (np.asarray(w, f) for w in (wq, wk, wv, wo))
    w1_, w2_ = np.asarray(w1, f), np.asarray(w2, f)
    gam, bet = np.asarray(gamma, f), np.asarray(beta, f)
    bo_, b1_, b2_ = np.asarray(bo, f), np.asarray(b1, f), np.asarray(b2, f)

    # gamma folds into weight columns; beta becomes per-output biases
    bq = wq_ @ bet
    bk = wk_ @ bet
    bv = wv_ @ bet
    bo_eff = bo_ + wo_ @ bv
    b1_eff = b1_ + w1_ @ bet

    common = {
        "wqT": np.ascontiguousarray((wq_.T * gam[:, None]).astype(bf)),
        "wkT": np.ascontiguousarray((wk_.T * gam[:, None]).astype(bf)),
        "wvT": np.ascontiguousarray((wv_.T * gam[:, None]).astype(bf)),
        "woT": np.ascontiguousarray(wo_.T.astype(bf)),
        "w1T": np.ascontiguousarray((w1_.T * gam[:, None]).astype(bf)),
        "w2T": np.ascontiguousarray(w2_.T.astype(bf)),
        "b1rs": np.ascontiguousarray(b1_eff.reshape(FK, 128).T),
        "bo_row": bo_eff.reshape(1, E),
        "b2_row": b2_.reshape(1, E),
        "bqT": np.ascontiguousarray(bq.reshape(EK, 128).T),
        "bkT": np.ascontiguousarray(bk.reshape(EK, 128).T),
    }
    # multiplicative diagonal-slot masks [k, q]:
    # r<j -> 1, r==j -> tri(k<=q), r>j -> 0
    kidx = np.arange(128)[:, None]
    qidx = np.arange(128)[None, :]
    tri = (kidx <= qidx).astype(f)
    in_maps = []
    for c in range(NCORES):
        b, j = c // GROUP, c % GROUP
        tiles = [j, j + 4, j + 8, j + 12]
        m = dict(common)
        m["x_own"] = np.ascontiguousarray(
            np.concatenate([xf[b, t * 128:(t + 1) * 128, :] for t in tiles]))
        mk = np.zeros((GROUP, 128, 128), f)
        mk[0:j] = 1.0
        mk[j] = tri
        m["masks"] = mk.astype(bf)
        in_maps.append(m)
    return in_maps


def run(in_maps, trace=False):
    nc = _get_module()
    return run_bass_kernel_spmd(nc, in_maps, core_ids=list(range(NCORES)),
                                trace=trace)


def kernel(x, wq, wk, wv, wo, bo, w1, b1, w2, b2, gamma, beta):
    in_maps = make_in_maps(x, wq, wk, wv, wo, bo, w1, b1, w2, b2, gamma, beta)
    res = run(in_maps)
    out = np.zeros((B, S, E), np.float32)
    for c in range(NCORES):
        b, j = c // GROUP, c % GROUP
        chunk = res.results[c]["out"]
        for slot, t in enumerate([j, j + 4, j + 8, j + 12]):
            out[b, t * 128:(t + 1) * 128] = chunk[slot * 128:(slot + 1) * 128]
    return out
